# revision 4
# baseline (speedup 1.0000x reference)
"""Bass/Trainium2 kernel for a variational (Bayes) LSTM.

Problem: T=512, B=32, I=H=1024.
    gates_x = einsum('tbi,gi->tbg', seq * mask_x, W_ih) + b_ih + b_hh
    step:    gates = gates_x[t] + (h * mask_h) @ W_hh.T
             i,f,g,o = split(gates); c = sig(f)*c + sig(i)*tanh(g); h = sig(o)*tanh(c)
Returns (output[T,B,H], (h,c)).

Sharding: gate/hidden dim 8-way (core c owns H-units [c*128,(c+1)*128) and their
four gate blocks). State kept transposed [h_unit(=partition), batch]. Each step:
  - 4 gate-blocks x 8 k-chunks of bf16 matmul vs the full masked-h [1024,32]
  - LSTM elementwise in [128, 4x32] layout (partitions = h units)
  - AllGather of the core's masked-h chunk [128,32] across the 8 cores
The input GEMM (gates_x) is emitted as paced filler between recurrence steps so
it runs inside the per-step AllGather gaps (keeps the PE warm, costs ~0 wall).
"""

import numpy as np
import ml_dtypes

import concourse.bass as bass
import concourse.bacc as bacc
import concourse.mybir as mybir
import concourse.tile as tile
from concourse.bass_utils import run_bass_kernel_spmd

T, B, I, H = 512, 32, 1024, 1024
NC = 8
HC = H // NC            # 128 hidden units per core
GC = 4 * HC             # 512 gate rows per core
KP = 128                # contraction chunk (partition dim)
NK = I // KP            # 8 k-chunks
PH1_N = 512             # phase-1 moving free dim per matmul
F32 = mybir.dt.float32
BF16 = mybir.dt.bfloat16
CDT = BF16              # matmul compute dtype
AF = mybir.ActivationFunctionType

BF16_NP = ml_dtypes.bfloat16


def build_nc(t_steps: int = T):
    """Build the SPMD Bass program (identical on all 8 cores)."""
    TBp = t_steps * B
    NB = TBp // PH1_N           # phase-1 tb blocks (each = PH1_N // B = 16 steps)
    steps_per_nb = PH1_N // B   # 16
    n_chunks = 4 * NB           # phase-1 (nb, m) chunks
    head = min(3 * 4, n_chunks)  # prologue: 3 blocks worth of chunks

    nc = bacc.Bacc(None, target_bir_lowering=False, num_devices=NC)

    # ---- I/O ----
    seqT = nc.declare_dram_parameter("seqT", [I, TBp], CDT, isOutput=False)
    wihT = nc.declare_dram_parameter("wihT", [I, GC], CDT, isOutput=False)
    whhT = nc.declare_dram_parameter("whhT", [H, GC], CDT, isOutput=False)
    biasP = nc.declare_dram_parameter("bias", [4, HC], F32, isOutput=False)
    mhT = nc.declare_dram_parameter("mhT", [HC, B], F32, isOutput=False)
    h0mT = nc.declare_dram_parameter("h0mT", [H, B], CDT, isOutput=False)
    c0T = nc.declare_dram_parameter("c0T", [HC, B], F32, isOutput=False)
    outT = nc.declare_dram_parameter("outT", [t_steps, HC, B], F32, isOutput=True)
    cTf = nc.declare_dram_parameter("cTf", [HC, B], F32, isOutput=True)

    rg = [list(range(NC))]

    with tile.TileContext(nc) as tc:
        with (
            tc.tile_pool(name="const", bufs=1) as constp,
            tc.tile_pool(name="seq", bufs=16) as seqp,
            tc.tile_pool(name="ps1", bufs=4, space="PSUM") as ps1p,
            tc.tile_pool(name="evac", bufs=3) as evacp,
            tc.tile_pool(name="gxd", bufs=4, space="DRAM") as gxdp,
            tc.tile_pool(name="gx8", bufs=3) as gx8p,
            tc.tile_pool(name="ps2", bufs=2, space="PSUM") as ps2p,
            tc.tile_pool(name="hm", bufs=2) as hmp,
            tc.tile_pool(name="agio", bufs=4, space="DRAM") as agiop,
            tc.tile_pool(name="ew", bufs=2) as ewp,
            tc.tile_pool(name="st", bufs=1) as stp,
        ):
            # ---- resident constants ----
            wih_sb = constp.tile([KP, NK, GC], CDT, tag="wih")
            nc.sync.dma_start(wih_sb[:], wihT[:].rearrange("(n p) g -> p n g", p=KP))
            whh_sb = constp.tile([KP, NK, GC], CDT, tag="whh")
            nc.sync.dma_start(whh_sb[:], whhT[:].rearrange("(n p) g -> p n g", p=KP))
            bias_sb = constp.tile([HC, 4], F32, tag="bias")
            nc.sync.dma_start(bias_sb[:], biasP[:].rearrange("m p -> p m"))
            mh_sb = constp.tile([HC, B], F32, tag="mh")
            nc.sync.dma_start(mh_sb[:], mhT[:])

            cT_sb = stp.tile([HC, B], F32, tag="cT")
            nc.sync.dma_start(cT_sb[:], c0T[:])

            # ---- phase-1 emitter: one (nb, m) chunk of the gates_x GEMM ----
            gx_tiles = [None] * NB
            seq_cur = [None] * NK

            def emit_ph1_chunk(ci: int):
                nb, m = divmod(ci, 4)
                if m == 0:
                    gx_tiles[nb] = gxdp.tile([4, HC, PH1_N], F32, tag="gx",
                                             name=f"gx{nb}")
                    for k in range(NK):
                        sq = seqp.tile([KP, PH1_N], CDT, tag="sq", name=f"sq{k}")
                        nc.sync.dma_start(
                            sq[:],
                            seqT[k * KP:(k + 1) * KP, nb * PH1_N:(nb + 1) * PH1_N],
                        )
                        seq_cur[k] = sq
                ps = ps1p.tile([HC, PH1_N], F32, tag="ps1")
                for k in range(NK):
                    nc.tensor.matmul(
                        ps[:],
                        wih_sb[:, k, m * HC:(m + 1) * HC],
                        seq_cur[k][:],
                        start=(k == 0),
                        stop=(k == NK - 1),
                    )
                ev = evacp.tile([HC, PH1_N], F32, tag="ev")
                nc.scalar.activation(ev[:], ps[:], AF.Identity, bias=bias_sb[:, m:m + 1])
                nc.sync.dma_start(gx_tiles[nb][m], ev[:])

            emitted = 0
            while emitted < head:
                emit_ph1_chunk(emitted)
                emitted += 1

            # ---- phase 2: the recurrence ----
            hm_t = hmp.tile([KP, NK, B], CDT, tag="hm")
            nc.sync.dma_start(hm_t[:], h0mT[:].rearrange("(n p) b -> p n b", p=KP))

            gx8_t = None
            for t in range(t_steps):
                nb, r = divmod(t, steps_per_nb)
                if r % 8 == 0:
                    j8 = r // 8
                    gx8_t = gx8p.tile([HC, 4, 8 * B], F32, tag="gx8")
                    nc.sync.dma_start(
                        gx8_t[:],
                        gx_tiles[nb][:, :, j8 * 8 * B:(j8 + 1) * 8 * B]
                        .rearrange("m p x -> p m x"),
                    )
                j = r % 8

                # gates chunk: [128 h, 4 gates, 32 b] = W_hh-part + gx
                gps = ps2p.tile([HC, 4, B], F32, tag="gps")
                for m in range(4):
                    for k in range(NK):
                        nc.tensor.matmul(
                            gps[:, m, :],
                            whh_sb[:, k, m * HC:(m + 1) * HC],
                            hm_t[:, k, :],
                            start=(k == 0),
                            stop=(k == NK - 1),
                        )
                gsb = ewp.tile([HC, 4, B], F32, tag="gsb")
                nc.vector.tensor_add(gsb[:], gps[:], gx8_t[:, :, j * B:(j + 1) * B])

                sig = ewp.tile([HC, 3, B], F32, tag="sig")
                nc.scalar.activation(sig[:], gsb[:, 0:3, :], AF.Sigmoid)
                tg = ewp.tile([HC, B], F32, tag="tg")
                nc.scalar.activation(tg[:], gsb[:, 3, :], AF.Tanh)

                t1 = ewp.tile([HC, B], F32, tag="t1")
                nc.vector.tensor_mul(t1[:], sig[:, 1, :], cT_sb[:])   # f*c
                t2 = ewp.tile([HC, B], F32, tag="t2")
                nc.vector.tensor_mul(t2[:], sig[:, 0, :], tg[:])      # i*tanh(g)
                nc.vector.tensor_add(cT_sb[:], t1[:], t2[:])
                tc_t = ewp.tile([HC, B], F32, tag="tc")
                nc.scalar.activation(tc_t[:], cT_sb[:], AF.Tanh)
                hT = ewp.tile([HC, B], F32, tag="hT")
                nc.vector.tensor_mul(hT[:], sig[:, 2, :], tc_t[:])    # o*tanh(c)

                nc.sync.dma_start(outT[t], hT[:])

                if t < t_steps - 1:
                    hmo = ewp.tile([HC, B], CDT, tag="hmo")
                    nc.vector.tensor_mul(hmo[:], hT[:], mh_sb[:])
                    ag_in = agiop.tile([HC, B], CDT, tag="agin")
                    nc.sync.dma_start(ag_in[:], hmo[:])
                    ag_out = agiop.tile([H, B], CDT, tag="agout")
                    nc.gpsimd.collective_compute(
                        "AllGather",
                        mybir.AluOpType.bypass,
                        replica_groups=rg,
                        ins=[ag_in.opt()],
                        outs=[ag_out.opt()],
                    )
                    hm_t = hmp.tile([KP, NK, B], CDT, tag="hm")
                    nc.sync.dma_start(
                        hm_t[:], ag_out[:].rearrange("(n p) b -> p n b", p=KP)
                    )

                # paced phase-1 filler: 1 chunk per 4 steps keeps the gates_x
                # GEMM inside the AllGather gaps without backpressure
                if t % 4 == 3 and emitted < n_chunks:
                    emit_ph1_chunk(emitted)
                    emitted += 1

            while emitted < n_chunks:
                emit_ph1_chunk(emitted)
                emitted += 1

            nc.sync.dma_start(cTf[:], cT_sb[:])

    nc.compile()
    return nc


def _rows(c: int) -> np.ndarray:
    """Gate rows of core c in [i, f, o, g] block order."""
    u = np.arange(c * HC, (c + 1) * HC)
    return np.concatenate([u, H + u, 3 * H + u, 2 * H + u])


def make_in_maps(sequence, h0, c0, mask_x, mask_h, W_ih, W_hh, b_ih, b_hh,
                 t_steps: int = T):
    seq = np.asarray(sequence, np.float32)[:t_steps]
    h0 = np.asarray(h0, np.float32)
    c0 = np.asarray(c0, np.float32)
    mask_x = np.asarray(mask_x, np.float32)
    mask_h = np.asarray(mask_h, np.float32)
    W_ih = np.asarray(W_ih, np.float32)
    W_hh = np.asarray(W_hh, np.float32)
    bsum = (np.asarray(b_ih, np.float32) + np.asarray(b_hh, np.float32))

    sm = (seq * mask_x[None, :, :]).transpose(2, 0, 1).reshape(I, t_steps * B)
    sm = np.ascontiguousarray(sm).astype(BF16_NP)
    h0m = np.ascontiguousarray((h0 * mask_h).T).astype(BF16_NP)

    in_maps = []
    for c in range(NC):
        rows = _rows(c)
        in_maps.append({
            "seqT": sm,
            "wihT": np.ascontiguousarray(W_ih[rows].T).astype(BF16_NP),
            "whhT": np.ascontiguousarray(W_hh[rows].T).astype(BF16_NP),
            "bias": np.ascontiguousarray(bsum[rows].reshape(4, HC)),
            "mhT": np.ascontiguousarray(mask_h[:, c * HC:(c + 1) * HC].T),
            "h0mT": h0m,
            "c0T": np.ascontiguousarray(c0[:, c * HC:(c + 1) * HC].T),
        })
    return in_maps


def assemble(results, t_steps: int = T):
    out = np.empty((t_steps, B, H), np.float32)
    c_fin = np.empty((B, H), np.float32)
    for c in range(NC):
        o = np.asarray(results[c]["outT"]).reshape(t_steps, HC, B)
        out[:, :, c * HC:(c + 1) * HC] = o.transpose(0, 2, 1)
        c_fin[:, c * HC:(c + 1) * HC] = np.asarray(results[c]["cTf"]).reshape(HC, B).T
    h_fin = out[-1].copy()
    return out, (h_fin, c_fin)


_NC_CACHE = {}


def kernel(**inputs):
    if T not in _NC_CACHE:
        _NC_CACHE[T] = build_nc(T)
    nc = _NC_CACHE[T]
    in_maps = make_in_maps(**inputs)
    res = run_bass_kernel_spmd(nc, in_maps, core_ids=list(range(NC)))
    return assemble(res.results)
